# revision 21
# baseline (speedup 1.0000x reference)
"""Bidirectional ConvLSTM block for Trainium2 (Bass/Tile), 8-core SPMD.

Problem: x [S=16, B=4, Cin=32, H=128, W=128] f32, Wf/Wb [128, 64, 3, 3],
bf/bb [128].  Output [S, B, 2*Co=64, H, W]: forward ConvLSTM hidden states
concat backward ConvLSTM (run on time-reversed x, not re-flipped).

Sharding: 8 independent recurrences = 2 directions x 4 batch elements.
Core k runs direction d=k//4 on batch b=k%4.  No cross-core communication.

Per-core kernel design (V2):
  - Activation planes have NO column padding: each plane is 130 rows x 128
    cols (rows 0/129 zero).  Column shifts are materialized as separate
    partition-group copies so every matmul rhs is a flat contiguous
    [128, 512] slice and every h-writeback DMA is a contiguous 1KB/partition
    run (vs strided 256B descriptors in V1).
  - Two plane tiles (ping/pong x2):
      A: [xC 0:32 | hC 32:64 | xL 64:96 | hL 96:128]
      B: [xR 0:32 | hR 32:64 | xRd 64:96 | hRd 96:128]
    C = centered, L = left-shift (tap dx=+1), R = right-shift (dx=-1),
    Rd = right-shift + down-shift (tap (dy-1, dx=-1)).
  - 5 matmul passes per gate cover all 9 conv taps (vs 6 in V1):
      P0-P2: tile A at dy=-1,0,+1 -> taps (dy,0) + (dy,+1), K=128
      P3:    tile B at dy=0       -> taps (0,-1) + (-1,-1), K=128
      P4:    tile B at dy=+1      -> tap  (+1,-1),          K=64
  - Gates col-tiled M=32 (tile_position (0,32j)): 4 spatial tiles stacked
    into [128, 2048] psum tiles (4 banks), one psum tile per group of 16
    image rows; bufs=2 -> next group's matmuls only WAR-wait on a
    2-groups-ago pointwise (no step-boundary PE drain).
  - Pointwise in fp16 (2x DVE), c-state fp16.  tanh(c)+h production for
    group g is emitted in group g+1's block so the ACT/DVE queues never
    ping-pong within a group.  h is produced once as bf16 (= y output)
    plus two shifted copies (DVE + gpsimd), then 4 contiguous DMAs place
    C/L/R/Rd copies for the next step.
"""

import os
import sys

import numpy as np

for _p in ("/opt/trn_rl_repo", "/root/.axon_site/_ro/trn_rl_repo"):
    if os.path.isdir(_p) and _p not in sys.path:
        sys.path.insert(0, _p)

import ml_dtypes  # noqa: E402
import concourse.bass as bass  # noqa: E402,F401
import concourse.mybir as mybir  # noqa: E402
from concourse import bacc, tile  # noqa: E402
from concourse.bass_utils import run_bass_kernel_spmd  # noqa: E402

F32 = mybir.dt.float32
F16 = mybir.dt.float16
BF16 = mybir.dt.bfloat16
AF = mybir.ActivationFunctionType

S, B, CIN, H, W = 16, 4, 32, 128, 128
CO = 32
HP = H + 2                     # 130 rows (rows 0/129 zero)
PLN = HP * W                   # 16640 elements per plane
NSP = H * W                    # 16384
NT = 512                       # spatial positions per matmul tile (4 rows)
TPG = 4                        # tiles per group
GROUPS = NSP // (NT * TPG)     # 8 groups per step; group = 16 image rows
NPASS = 5
N_CORES = 8


def build_kernel(nc, tc, x_ap, w_ap, b_ap, y_ap, z_ap, n_steps):
    ctx_pools = []

    def pool(**kw):
        p = tc.tile_pool(**kw)
        ctx_pools.append(p)
        return p.__enter__()

    const = pool(name="const", bufs=1)
    tmp = pool(name="tmp", bufs=3)
    psum = pool(name="psum", bufs=2, space="PSUM")

    # Persistent tiles
    A = [const.tile([128, PLN], BF16, tag=f"A{i}", name=f"A{i}") for i in range(2)]
    Bt = [const.tile([128, PLN], BF16, tag=f"B{i}", name=f"B{i}") for i in range(2)]
    ctile = const.tile([128, GROUPS * NT], F16, tag="c")
    wsb = const.tile([128, 4 * NPASS * 32], BF16, tag="w")
    bsb = const.tile([128, 4], F32, tag="bias")
    # h staging: one batch = BAT groups, so writeback DMAs merge 4 groups
    # per trigger (sequencer DIRECT2D descriptor-gen is ~600ns per trigger)
    BAT = 4
    hbf = [const.tile([128, BAT * NT], BF16, tag=f"hbf{i}", name=f"hbf{i}") for i in range(2)]
    hbfL = [const.tile([128, BAT * NT], BF16, tag=f"hbfL{i}", name=f"hbfL{i}") for i in range(2)]
    hbfR = [const.tile([128, BAT * NT], BF16, tag=f"hbfR{i}", name=f"hbfR{i}") for i in range(2)]

    nc.sync.dma_start(wsb[:, :], w_ap)
    nc.sync.dma_start(bsb[:, :], b_ap)

    # --- one-time zero init.  Parity-0 h regions are zero-filled from a
    # zero plane in HBM (overlaps the x load); parity-1 interiors are fully
    # rewritten during step 0, so they only need their pad rows zeroed.
    for tl in (A[0], Bt[0]):
        nc.sync.dma_start(tl[32:64, :], z_ap)
        nc.sync.dma_start(tl[96:128, :], z_ap)
    nc.vector.memset(A[1][32:64, 0:W], 0.0)              # hC row 0
    nc.vector.memset(A[1][32:64, (HP - 1) * W :], 0.0)   # hC row 129
    nc.vector.memset(A[1][96:128, 0:W], 0.0)             # hL row 0
    nc.vector.memset(A[1][96:128, (HP - 1) * W :], 0.0)
    nc.gpsimd.memset(Bt[1][32:64, 0:W], 0.0)             # hR row 0 (unread)
    nc.gpsimd.memset(Bt[1][32:64, (HP - 1) * W :], 0.0)  # hR row 129
    nc.gpsimd.memset(Bt[1][96:128, 0 : 2 * W], 0.0)      # hRd rows 0-1
    nc.vector.memset(ctile[:, :], 0.0)
    for tl in hbf + hbfL + hbfR:
        nc.gpsimd.memset(tl[:, :], 0.0)

    def load_x(t, half=None):
        # split into halves: finer descriptors interleave with h writebacks
        hn = PLN // 2
        lo, hi = (0, PLN) if half is None else (half * hn, half * hn + hn)
        nc.scalar.dma_start(A[t % 2][0:32, lo:hi], x_ap[t, 0, :, lo:hi])
        nc.scalar.dma_start(A[t % 2][64:96, lo:hi], x_ap[t, 1, :, lo:hi])
        nc.scalar.dma_start(Bt[t % 2][0:32, lo:hi], x_ap[t, 2, :, lo:hi])
        nc.scalar.dma_start(Bt[t % 2][64:96, lo:hi], x_ap[t, 3, :, lo:hi])

    load_x(0)

    # state carried between group blocks: (t, g, so, zt) awaiting tail emit
    pending = [None]

    def gates_block(t, g, zt):
        """ACT gate activations + DVE c-update for group g of step t."""
        csl = ctile[:, g * NT : (g + 1) * NT]
        sf = tmp.tile([128, NT], F16, tag="sf", name=f"sf{t}_{g}")
        si = tmp.tile([128, NT], F16, tag="si", name=f"si{t}_{g}")
        tg = tmp.tile([128, NT], F16, tag="tg", name=f"tg{t}_{g}")
        so = tmp.tile([128, NT], F16, tag="so", name=f"so{t}_{g}")
        nc.scalar.activation(sf[:, :], zt[:, NT : 2 * NT], AF.Sigmoid, bias=bsb[:, 1:2])
        nc.scalar.activation(si[:, :], zt[:, 0:NT], AF.Sigmoid, bias=bsb[:, 0:1])
        nc.scalar.activation(tg[:, :], zt[:, 3 * NT : 4 * NT], AF.Tanh, bias=bsb[:, 3:4])
        nc.scalar.activation(so[:, :], zt[:, 2 * NT : 3 * NT], AF.Sigmoid, bias=bsb[:, 2:3])
        t3 = tmp.tile([128, NT], F16, tag="t3", name=f"t3_{t}_{g}")
        t2 = tmp.tile([128, NT], F16, tag="t2", name=f"t2_{t}_{g}")
        nc.vector.tensor_mul(t3[:, :], sf[:, :], csl)
        nc.vector.tensor_mul(t2[:, :], si[:, :], tg[:, :])
        nc.vector.tensor_add(csl, t2[:, :], t3[:, :])
        return so

    def tail_compute(t, g, so):
        """tanh(c) + h production for group g of step t, staged into the
        batch tiles.  Emitted one group later so engines don't ping-pong."""
        csl = ctile[:, g * NT : (g + 1) * NT]
        par = (g // BAT) % 2
        gi = g % BAT
        tcn = tmp.tile([128, NT], F16, tag="tcn", name=f"tcn{t}_{g}")
        nc.scalar.activation(tcn[:, :], csl, AF.Tanh)
        hb = hbf[par][:, gi * NT : (gi + 1) * NT]
        nc.vector.tensor_mul(hb, so[:, :], tcn[:, :])
        # shifted copies: L[.., 0:127] = h[.., 1:128]; R[.., 1:128] = h[.., 0:127]
        hb3 = hb.rearrange("p (r w) -> p r w", r=TPG)
        hl3 = hbfL[par][:, gi * NT : (gi + 1) * NT].rearrange("p (r w) -> p r w", r=TPG)
        hr3 = hbfR[par][:, gi * NT : (gi + 1) * NT].rearrange("p (r w) -> p r w", r=TPG)
        nc.vector.tensor_copy(hl3[:, :, 0 : W - 1], hb3[:, :, 1:W])
        nc.gpsimd.tensor_copy(hr3[:, :, 1:W], hb3[:, :, 0 : W - 1])

    def batch_dma(t, bi):
        """Writeback DMAs for batch bi (groups 4bi..4bi+3) of step t: one
        trigger per (copy, j) covering 4 groups, plus one y trigger."""
        par = bi % 2
        nc.sync.dma_start(y_ap[t, bi], hbf[par][:, :])
        if t + 1 >= n_steps:
            return
        An, Bn = A[(t + 1) % 2], Bt[(t + 1) % 2]
        GRP = 16 * W  # 2048: plane elements per 16-row group
        for j in range(TPG):
            # src: [32, BAT, 512]; dst rows for group g'=4bi+k, tile j start
            # at plane row 16g'+4j+1 -> stride GRP between groups.  Strided
            # dst view: rearrange a window into 512-chunks, step-slice by 4.
            src = hbf[par][32 * j : 32 * j + 32, :].rearrange(
                "p (b f) -> p b f", b=BAT)
            srcL = hbfL[par][32 * j : 32 * j + 32, :].rearrange(
                "p (b f) -> p b f", b=BAT)
            srcR = hbfR[par][32 * j : 32 * j + 32, :].rearrange(
                "p (b f) -> p b f", b=BAT)
            base = BAT * GRP * bi + NT * j + W
            win = lambda tl, pl, b: tl[
                pl : pl + 32, b : b + (BAT - 1) * GRP + NT
            ].rearrange("p (b f) -> p b f", f=NT)[:, 0 :: GRP // NT, :]
            nc.sync.dma_start(win(An, 32, base), src)
            nc.sync.dma_start(win(An, 96, base), srcL)
            nc.sync.dma_start(win(Bn, 32, base), srcR)
            nc.sync.dma_start(win(Bn, 96, base + W), srcR)

    for t in range(n_steps):
        Ac, Bc = A[t % 2], Bt[t % 2]

        for g in range(GROUPS):
            zt = psum.tile([128, 4 * NT], F32, tag="z", name=f"z{t}_{g}")
            for gate in range(4):
                for p in range(NPASS):
                    col = (gate * NPASS + p) * 32
                    for j in range(TPG):
                        rr = 16 * g + 4 * j
                        if p < 3:
                            lo, hi, tl, o = 0, 128, Ac, (rr + p) * W
                        elif p == 3:
                            lo, hi, tl, o = 0, 128, Bc, (rr + 1) * W
                        else:
                            lo, hi, tl, o = 0, 64, Bc, (rr + 2) * W
                        nc.tensor.matmul(
                            zt[32 * j : 32 * j + 32, gate * NT : (gate + 1) * NT],
                            wsb[lo:hi, col : col + 32],
                            tl[lo:hi, o : o + NT],
                            start=(p == 0),
                            stop=(p == NPASS - 1),
                            skip_group_check=True,
                            tile_position=(0, 32 * j),
                        )
            so = gates_block(t, g, zt)
            if pending[0] is not None:
                pt, pg, pso = pending[0]
                tail_compute(pt, pg, pso)
                if pg % BAT == BAT - 1:
                    batch_dma(pt, pg // BAT)
                    # prefetch next x AFTER batch-0 h writes: the h
                    # writebacks preempt the bulky x descriptors in the
                    # DMA engine queues
                    if pg // BAT == 0 and pt + 1 < n_steps:
                        load_x(pt + 1, half=0)
            pending[0] = (t, g, so)
        # flush the last group at step end so batch-1 h writes land early
        pt, pg, pso = pending[0]
        tail_compute(pt, pg, pso)
        batch_dma(pt, pg // BAT)
        if pt + 1 < n_steps:
            load_x(pt + 1, half=1)
        pending[0] = None

    for p in reversed(ctx_pools):
        p.__exit__(None, None, None)


def build_program(n_steps=S):
    nc = bacc.Bacc(
        "TRN2",
        target_bir_lowering=False,
        debug=False,
        enable_asserts=False,
        num_devices=N_CORES,
    )
    x_d = nc.dram_tensor("x", [n_steps, 4, CIN, PLN], BF16, kind="ExternalInput")
    z_d = nc.dram_tensor("z0", [CIN, PLN], BF16, kind="ExternalInput")
    w_d = nc.dram_tensor("w", [128, 4 * NPASS * 32], BF16, kind="ExternalInput")
    b_d = nc.dram_tensor("bias", [128, 4], F32, kind="ExternalInput")
    y_d = nc.dram_tensor("y", [n_steps, 2, 128, 4 * NT], BF16, kind="ExternalOutput")
    with tile.TileContext(nc) as tc:
        build_kernel(nc, tc, x_d.ap(), w_d.ap(), b_d.ap(), y_d.ap(), z_d.ap(), n_steps)
    nc.compile()
    return nc


def pack_weights(Wd):
    """Wd [128, 64, 3, 3] f32 -> lhsT blocks [128, 20*32] bf16.

    Block (gate, p): P0-P2 -> [xC ky=p kx=1 | hC | xL kx=2 | hL],
    P3 -> [xR ky=1 kx=0 | hR | xRd ky=0 kx=0 | hRd], P4 -> [xR ky=2 kx=0 |
    hR | zeros]."""
    wp = np.zeros((128, 4 * NPASS, 32), np.float32)
    for g in range(4):
        Wg = Wd[g * 32 : (g + 1) * 32]  # [32(m), 64, 3, 3]
        for p in range(3):
            blk = wp[:, g * NPASS + p, :]
            blk[0:32] = Wg[:, 0:32, p, 1].T
            blk[32:64] = Wg[:, 32:64, p, 1].T
            blk[64:96] = Wg[:, 0:32, p, 2].T
            blk[96:128] = Wg[:, 32:64, p, 2].T
        blk = wp[:, g * NPASS + 3, :]
        blk[0:32] = Wg[:, 0:32, 1, 0].T
        blk[32:64] = Wg[:, 32:64, 1, 0].T
        blk[64:96] = Wg[:, 0:32, 0, 0].T
        blk[96:128] = Wg[:, 32:64, 0, 0].T
        blk = wp[:, g * NPASS + 4, :]
        blk[0:32] = Wg[:, 0:32, 2, 0].T
        blk[32:64] = Wg[:, 32:64, 2, 0].T
    return wp.reshape(128, 4 * NPASS * 32).astype(ml_dtypes.bfloat16)


def pack_bias(bd):
    """bd [128] f32 -> [128, 4] f32 (partition p = 32*tile + ch)."""
    bp = np.zeros((128, 4), np.float32)
    for g in range(4):
        bp[:, g] = np.tile(bd[g * 32 : (g + 1) * 32], 4)
    return bp


def pack_x(xc, n_steps):
    """xc [S, 32, 128, 128] f32 -> [S, 4, 32, PLN] bf16 (C, L, R, Rd)."""
    xp = np.zeros((n_steps, 4, CIN, HP, W), np.float32)
    xp[:, 0, :, 1 : H + 1, :] = xc
    xp[:, 1, :, 1 : H + 1, 0 : W - 1] = xc[..., 1:]
    xp[:, 2, :, 1 : H + 1, 1:W] = xc[..., : W - 1]
    xp[:, 3, :, 2 : H + 2, 1:W] = xc[..., : W - 1]
    return xp.reshape(n_steps, 4, CIN, PLN).astype(ml_dtypes.bfloat16)


def make_in_maps(x, Wf, bf, Wb, bb, n_steps=S):
    wpacks = [pack_weights(np.asarray(Wf, np.float32)),
              pack_weights(np.asarray(Wb, np.float32))]
    bpacks = [pack_bias(np.asarray(bf, np.float32)),
              pack_bias(np.asarray(bb, np.float32))]
    x = np.asarray(x, np.float32)
    in_maps = []
    for k in range(N_CORES):
        d, b = k // 4, k % 4
        xc = x[:n_steps, b] if d == 0 else x[::-1][:n_steps, b]
        in_maps.append(
            {
                "x": pack_x(xc, n_steps),
                "w": wpacks[d],
                "bias": bpacks[d],
                "z0": np.zeros((CIN, PLN), ml_dtypes.bfloat16),
            }
        )
    return in_maps


_CACHED_NC = None


def unpack_y(y):
    """y [S, 2, 128, 2048] (bi, 32j+ch, gi*512 + 4rows*128) -> [S, 32, H, W].

    Row of (bi, gi, j, r4) = 16*(4bi+gi) + 4j + r4."""
    yk = np.asarray(y, dtype=np.float32).reshape(S, 2, 4, 32, 4, 4, W)
    return yk.transpose(0, 3, 1, 4, 2, 5, 6).reshape(S, CO, H, W)


def kernel(x, Wf, bf, Wb, bb):
    global _CACHED_NC
    if _CACHED_NC is None:
        _CACHED_NC = build_program(S)
    nc = _CACHED_NC
    in_maps = make_in_maps(x, Wf, bf, Wb, bb)
    res = run_bass_kernel_spmd(nc, in_maps, core_ids=list(range(N_CORES)))
    out = np.empty((S, B, 2 * CO, H, W), np.float32)
    for k in range(N_CORES):
        d, b = k // 4, k % 4
        out[:, b, d * CO : (d + 1) * CO] = unpack_y(res.results[k]["y"])
    return out


if __name__ == "__main__":
    rng = np.random.default_rng(0)
    x = rng.standard_normal((S, B, CIN, H, W), np.float32)
    Wf = (rng.standard_normal((128, 64, 3, 3)) * 0.05).astype(np.float32)
    Wb = (rng.standard_normal((128, 64, 3, 3)) * 0.05).astype(np.float32)
    bf = np.zeros(128, np.float32)
    bb = np.zeros(128, np.float32)
    y = kernel(x, Wf, bf, Wb, bb)
    print("out", y.shape, y.dtype)
